# revision 53
# baseline (speedup 1.0000x reference)
"""STBlock (temporal attn -> spatial attn -> ChebConv + residual, relu) on 8 trn2 cores.

Sharding: data-parallel over batch B=8, one batch element per core.

v8 (~137us, from the 151-183us staged baseline). Structure on top of the
original fp8-DoubleRow design (all five N-contractions in fp8 DR, t-major
column order, banded E application, transposed output):
  - every input tensor is pre-blocked on the host so it loads as ONE
    dma_start (DMA issue costs ~0.7us each on the Sync queue regardless of
    size); small constants issue from the Act HWDGE queue; the first Gram
    matmuls start on the first xnp half.
  - spatial attention is computed TRANSPOSED and UNNORMALIZED: S_pre^T =
    Sig @ Vs^T (Sig is symmetric) + bs^T via identity matmuls; Act's Exp
    writes 4*exp(pre) straight to fp8 (bias=ln4 keeps the range in [0.13,120]
    under the 240 e4m3 max). Softmax denominators come from a DoubleRow
    ones-matmul column-sum; Z0 is then computed NATURAL (n on partitions) so
    the 1/colsum normalization rides the existing per-partition-scaled PSUM
    drains for free. The reciprocal runs on a [128,8] partition-layout tile
    (built by 8 PE transposes of the colsum) because DVE reciprocal cost is
    proportional to free-dim length (6.5us for [128,1024], 150ns here), and
    the whole chain overlaps the Z0 matmuls.
  - the Chebyshev recurrence uses a HOST-precomputed L2 = 2L^2 (the graph is
    shared): Z2' = L2 @ Z0 runs directly off z0nb, so the z1nb transpose pass
    and the Z2 subtract disappear, and W0' = W0 - W2 absorbs the -Z0 term
    into the projection weights. Z1 exists only as fp8 x512, drained straight
    from PSUM (psum scale == tail scale).
  - the tail projection pairs (Z0, Z1) as one fp8x512 DoubleRow matmul
    (3 matmuls per half instead of 4). All tail drains emit 512-scaled
    outputs (relu commutes with positive scale); the host divides by 512.

Engine traps (measured): Pool writes fp8 at 14.7us/tile (never do it); DVE
fp8-out tensor ops are ~2.4x bf16-out; Act writes fp8 at full speed; a matmul
output must not cross a PSUM bank (512 fp32 cols); the Act function table
holds ONE function (each Sigmoid/Exp/Relu switch costs 1.28us, Copy exempt).

Layouts: d = f*24+t (f-major), d' = t*32+f (t-major); out^T row = t*64+g.
fp8 scales: x direct, satb 4*exp, Z0/Z1 x512 (tail) and x64 (L-contraction),
L^T x8, 2(L^2)^T x64. PSUM: psb 2x2 + pst 2x1 + ps1 1x2 = 8 banks.
"""
import numpy as np

B, N, F, T, G = 8, 1024, 32, 24, 64
D = F * T            # 768
NCH = N // 128       # 8 n-chunks
DCH = D // 128       # 6 d-tiles
QO = 12              # out^T tiles (1536 rows)
S2S = 64.0           # host scale on 2*L^2

_compiled = {}


def _build():
    if "nc" in _compiled:
        return _compiled["nc"]
    import concourse.mybir as mybir
    import concourse.bacc as bacc
    from concourse import tile

    FP = mybir.dt.float32
    BF = mybir.dt.bfloat16
    F8 = mybir.dt.float8e4
    AF = mybir.ActivationFunctionType
    OP = mybir.AluOpType
    DR = mybir.MatmulPerfMode.DoubleRow

    nc = bacc.Bacc("TRN2", target_bir_lowering=False, debug=False)

    xnp_d = nc.dram_tensor("xnp", (128, NCH * 1024), F8, kind="ExternalInput").ap()
    xt8f_d = nc.dram_tensor("xt8f", (128, DCH * N), F8, kind="ExternalInput").ap()
    bigi_d = nc.dram_tensor("bigi", (128, 384), BF, kind="ExternalInput").ap()
    xtt_d = nc.dram_tensor("xtt", (128, DCH * N), BF, kind="ExternalInput").ap()
    identb_d = nc.dram_tensor("identb", (128, 128), BF, kind="ExternalInput").ap()
    ident8_d = nc.dram_tensor("ident8", (128, 128), F8, kind="ExternalInput").ap()
    vetb_d = nc.dram_tensor("vetb", (T, T), BF, kind="ExternalInput").ap()
    be_d = nc.dram_tensor("be", (T, T), BF, kind="ExternalInput").ap()
    vst_d = nc.dram_tensor("vst8", (128, NCH * N), F8, kind="ExternalInput").ap()
    bst_d = nc.dram_tensor("bst", (128, NCH * N), BF, kind="ExternalInput").ap()
    lt8_d = nc.dram_tensor("lt8", (128, NCH * N), F8, kind="ExternalInput").ap()
    l2t8_d = nc.dram_tensor("l2t8", (128, NCH * N), F8, kind="ExternalInput").ap()
    wq8_d = nc.dram_tensor("wq8", (128, QO * 2 * 128), F8, kind="ExternalInput").ap()
    wpb2_d = nc.dram_tensor("wpb2", (128, QO * 2 * 128), BF, kind="ExternalInput").ap()
    bias_d = nc.dram_tensor("bias512", (128, 1), FP, kind="ExternalInput").ap()
    out_d = nc.dram_tensor("out", (QO * 128, N), BF, kind="ExternalOutput").ap()

    with tile.TileContext(nc) as tc:
        with (
            tc.tile_pool(name="persist", bufs=1) as pp,
            tc.tile_pool(name="stream", bufs=1) as sp,
            tc.tile_pool(name="psb", bufs=2, space="PSUM") as psb,
            tc.tile_pool(name="pst", bufs=2, space="PSUM") as pst,
            tc.tile_pool(name="ps1", bufs=1, space="PSUM") as ps1,
        ):
            _rr = [0]
            PSUM_SPACE = tile.bass.MemorySpace.PSUM

            def copy_rr(dst, src, engines=None):
                if engines is None:
                    if src.space == PSUM_SPACE or dst.space == PSUM_SPACE:
                        engines = (nc.vector, nc.scalar)
                    else:
                        engines = (nc.vector, nc.gpsimd)
                e = engines[_rr[0] % len(engines)]
                _rr[0] += 1
                if e is nc.scalar:
                    nc.scalar.activation(dst, src, AF.Copy)
                else:
                    e.tensor_copy(dst, src)

            def scaled_rr(dst, src, scale):
                if _rr[0] % 2 == 0:
                    nc.vector.tensor_scalar_mul(dst, src, scale)
                else:
                    nc.scalar.activation(dst, src, AF.Copy, scale=scale)
                _rr[0] += 1

            # ---- head DMAs: x first (S1 needs it; split in 2 so the first
            # Gram pairs can start on the first half), small constants go out
            # on the Act HWDGE queue to unserialize Sync issue.
            xnpb = pp.tile([128, NCH * 1024], F8, tag="xnpb")
            nc.sync.dma_start(xnpb[:, 0:4 * 1024], xnp_d[:, 0:4 * 1024])
            nc.sync.dma_start(xnpb[:, 4 * 1024:], xnp_d[:, 4 * 1024:])
            xnpbv = xnpb[:].rearrange("q (m c) -> q m c", m=NCH)
            vetb = pp.tile([T, T], BF, tag="vetb")
            nc.scalar.dma_start(vetb[:], vetb_d[:])
            be = pp.tile([T, T], BF, tag="be")
            nc.scalar.dma_start(be[:], be_d[:])
            bigi = pp.tile([128, 384], BF, tag="bigi")
            nc.scalar.dma_start(bigi[:], bigi_d[:])
            xt8f = pp.tile([128, DCH * N], F8, tag="xt8f")
            nc.sync.dma_start(xt8f[:], xt8f_d[:])
            xt8fv = xt8f[:].rearrange("q (p n) -> q p n", p=DCH)
            identb = pp.tile([128, 128], BF, tag="identb")
            nc.sync.dma_start(identb[:], identb_d[:])
            ident8 = pp.tile([128, 128], F8, tag="ident8")
            nc.sync.dma_start(ident8[:], ident8_d[:])
            vst8 = pp.tile([128, NCH * N], F8, tag="vst8")
            nc.sync.dma_start(vst8[:], vst_d[:])
            vst8v = vst8[:].rearrange("q (m n) -> q m n", m=NCH)
            bst = pp.tile([128, NCH * N], BF, tag="bst")
            nc.sync.dma_start(bst[:], bst_d[:])
            ones8 = pp.tile([128, 256], F8, tag="ones8")
            nc.gpsimd.memset(ones8[:], 1.0)
            ones8v = ones8[:].rearrange("q (k c) -> q k c", k=2)

            # ---- S1: score_t = sum_{n,f} x[n,f,t] x[n,f,u] ----
            # all 32 Gram matmuls accumulate into ONE psum group; the per-f
            # 24x24 diagonal blocks (32-aligned offsets) sum in place.
            # a2 outer so the first 16 matmuls need only the first xnp DMA
            pt = ps1.tile([128, 128], FP, tag="st")
            for a2 in range(4):  # DoubleRow over n-chunk pairs
                for g2 in range(8):
                    sl = xnpbv[:, 2 * a2:2 * a2 + 2,
                               g2 * 128:(g2 + 1) * 128]
                    nc.tensor.matmul(pt[:], sl, sl,
                                     start=(g2 == 0 and a2 == 0),
                                     stop=(g2 == 7 and a2 == 3),
                                     perf_mode=DR)
            dg = []
            for j in (1, 2, 3):
                t_ = sp.tile([T, T], FP, name=f"dg{j}", tag=f"dg{j}")
                nc.vector.tensor_copy(t_[:], pt[32 * j:32 * j + 24,
                                                32 * j:32 * j + 24])
                dg.append(t_)
            sct_a = sp.tile([T, T], FP, tag="sct_a")
            nc.vector.tensor_tensor(sct_a[:], pt[0:24, 0:24],
                                    dg[0][:], op=OP.add)
            sct_b = sp.tile([T, T], FP, tag="sct_b")
            nc.gpsimd.tensor_tensor(sct_b[:], dg[1][:], dg[2][:], op=OP.add)
            score_t = sp.tile([T, T], FP, tag="score_t")
            nc.vector.tensor_tensor(score_t[:], sct_a[:], sct_b[:], op=OP.add)

            # ---- S2: E_att = softmax(Ve @ sigmoid(score_t) + be) ----
            # be is folded in via an identity matmul; the normalize writes
            # straight into e4r (one less hop in this serial chain).
            sigb = sp.tile([T, T], BF, tag="sigb")
            nc.scalar.activation(sigb[:], score_t[:], AF.Sigmoid)
            ps_e = ps1.tile([T, T], FP, tag="st")
            nc.tensor.matmul(ps_e[:], vetb[:], sigb[:], start=True, stop=False)
            nc.tensor.matmul(ps_e[:], identb[0:T, 0:T], be[:],
                             start=False, stop=True)
            eexp = sp.tile([T, T], FP, tag="eexp")
            esum = sp.tile([T, 1], FP, tag="esum")
            nc.scalar.activation(eexp[:], ps_e[:], AF.Exp, accum_out=esum[:])
            einv = sp.tile([T, 1], FP, tag="einv")
            nc.vector.reciprocal(einv[:], esum[:])

            # EBIG: banded blocks of blockdiag(E_att x32), built on the PE
            bands = []
            for p in range(DCH):
                qs = []
                for q in (p - 1, p, p + 1):
                    if not 0 <= q < DCH:
                        continue
                    fs = [f for f in range(F)
                          if 24 * f < 128 * q + 128 and 24 * f + 24 > 128 * q
                          and 24 * f < 128 * p + 128 and 24 * f + 24 > 128 * p]
                    if fs:
                        qs.append((q, fs))
                bands.append(qs)
            soff = {}
            s = 0
            for p in range(DCH):
                for q, _ in bands[p]:
                    soff[(p, q)] = s
                    s += 1
            NB = s  # 14 blocks
            e4r = pp.tile([128, T], BF, tag="e4r")
            nc.gpsimd.memset(e4r[:], 0.0)
            nc.vector.tensor_scalar_mul(e4r[0:24, :], eexp[:], einv[:])
            ebig = pp.tile([128, NB * 128], F8, tag="ebig")
            nc.gpsimd.memset(ebig[:], 0.0)
            for half in range(2):
                blo = half * 7
                bhi = min(NB, blo + 7)
                pe_b = psb.tile([128, N], FP, tag="big")
                ranges = {}
                for p in range(DCH):
                    for q, fs in bands[p]:
                        sb = soff[(p, q)]
                        if not blo <= sb < bhi:
                            continue
                        for f in fs:
                            dlt = 24 * f - 128 * q
                            c0 = 24 * f - 128 * p
                            t0, t1 = max(0, -c0), min(24, 128 - c0)
                            cc = (sb - blo) * 128 + c0 + t0
                            nc.tensor.matmul(
                                pe_b[:, cc:cc + (t1 - t0)],
                                bigi[:, 128 - dlt:256 - dlt],
                                e4r[:, t0:t1], start=True, stop=True)
                            lo, hi = ranges.get(sb, (10 ** 9, -1))
                            ranges[sb] = (min(lo, c0 + t0), max(hi, c0 + t1))
                for sb, (lo, hi) in sorted(ranges.items()):
                    copy_rr(ebig[:, sb * 128 + lo:sb * 128 + hi],
                            pe_b[:, (sb - blo) * 128 + lo:(sb - blo) * 128 + hi])

            # ---- S3: TT8 = x_TA^T (f-major) via banded fp8 matmul ----
            tt8 = pp.tile([128, DCH * N], F8, tag="tt8")
            for p in range(DCH):
                pb = psb.tile([128, N], FP, tag="big")
                qs = bands[p]
                q0 = qs[0][0]
                s0 = soff[(p, q0)]
                for h in range(2):
                    nc.tensor.matmul(
                        pb[:, h * 512:(h + 1) * 512],
                        ebig[:, s0 * 128:(s0 + 2) * 128].rearrange(
                            "q (k c) -> q k c", k=2),
                        xt8fv[:, q0:q0 + 2, h * 512:(h + 1) * 512],
                        start=True, stop=(len(qs) == 2), perf_mode=DR)
                    if len(qs) == 3:
                        q2 = qs[2][0]
                        s2 = soff[(p, q2)]
                        nc.tensor.matmul(
                            pb[:, h * 512:(h + 1) * 512],
                            ebig[:, s2 * 128:(s2 + 1) * 128],
                            xt8f[:, q2 * N + h * 512:q2 * N + (h + 1) * 512],
                            start=False, stop=True)
                copy_rr(tt8[:, p * N:(p + 1) * N], pb[:])
            tt8v = tt8[:].rearrange("q (p n) -> q p n", p=DCH)

            # ---- S5: score_s -> sigmoid (sg8) interleaved with AN build ----
            sg8 = pp.tile([128, NCH * N], F8, tag="sg8")
            sg8v = sg8[:].rearrange("q (m n) -> q m n", m=NCH)
            anb = pp.tile([128, NCH * D], F8, tag="anb")
            anbv = anb[:].rearrange("q (m d) -> q m d", m=NCH)
            for i in range(NCH):
                pb = psb.tile([128, N], FP, tag="big")
                for h in range(2):
                    for a2 in range(3):  # DoubleRow over d-tile pairs
                        nc.tensor.matmul(
                            pb[:, h * 512:(h + 1) * 512],
                            tt8v[:, 2 * a2:2 * a2 + 2, i * 128:(i + 1) * 128],
                            tt8v[:, 2 * a2:2 * a2 + 2, h * 512:(h + 1) * 512],
                            start=(a2 == 0), stop=(a2 == 2), perf_mode=DR)
                nc.scalar.activation(sg8[:, i * N:(i + 1) * N], pb[:], AF.Sigmoid)

                # fp8 transpose must write psum with element step 2
                pa = pst.tile([128, 2 * D], F8, tag="tr")
                pav = pa[:].rearrange("q (c two) -> q two c", two=2)
                for p in range(DCH):
                    nc.tensor.transpose(pav[:, 0, p * 128:(p + 1) * 128],
                                        tt8[:, p * N + i * 128:p * N + (i + 1) * 128],
                                        ident8[:])
                dstv = anb[:, i * D:(i + 1) * D].rearrange(
                    "q (t f) -> q t f", t=T, f=F)
                srcv = pa[:].rearrange("q (f t two) -> q t f two",
                                       f=F, t=T, two=2)
                nc.vector.tensor_copy(dstv[:, 0:12, :].unsqueeze(3),
                                      srcv[:, 0:12, :, 0:1])
                nc.scalar.activation(dstv[:, 12:24, :].unsqueeze(3),
                                     srcv[:, 12:24, :, 0:1], AF.Copy)

            # ---- S6': S_att^T UNNORMALIZED: S_pre^T = Sig @ Vs^T + bs^T ----
            # (Sig symmetric). satb_un = 4*exp(pre^T) fp8 straight off the
            # Act engine (bias=ln4). Softmax denominators come from a
            # ones-matmul colsum; normalization is deferred to the S8 drains
            # where n sits on the PARTITION dim (per-partition scale = free).
            ln4 = pp.tile([128, 1], FP, tag="ln4")
            nc.gpsimd.memset(ln4[:], 1.3862943611198906)
            satb = pp.tile([128, NCH * N], F8, tag="satb")
            satbv = satb[:].rearrange("q (m n) -> q m n", m=NCH)
            cs = ps1.tile([128, N], FP, tag="st")
            for m in range(NCH):
                pb = psb.tile([128, N], FP, tag="big")
                for h in range(2):
                    for a2 in range(4):  # DoubleRow over k-chunk pairs
                        nc.tensor.matmul(
                            pb[:, h * 512:(h + 1) * 512],
                            sg8v[:, 2 * a2:2 * a2 + 2, m * 128:(m + 1) * 128],
                            vst8v[:, 2 * a2:2 * a2 + 2, h * 512:(h + 1) * 512],
                            start=(a2 == 0), stop=False, perf_mode=DR)
                    nc.tensor.matmul(
                        pb[:, h * 512:(h + 1) * 512],
                        identb[:], bst[:, m * N + h * 512:m * N + (h + 1) * 512],
                        start=False, stop=True)
                nc.scalar.activation(satb[:, m * N:(m + 1) * N], pb[:],
                                     AF.Exp, bias=ln4[:])
                if m % 2 == 1:  # DoubleRow colsum over satb chunk pairs
                    for h in range(2):
                        nc.tensor.matmul(
                            cs[:, h * 512:(h + 1) * 512], ones8v,
                            satbv[:, m - 1:m + 1, h * 512:(h + 1) * 512],
                            start=(m == 1), stop=(m == NCH - 1),
                            perf_mode=DR)
            # denominators -> partition layout: drain, 8 PE transposes, tiny
            # per-column reciprocals (8 elems/lane instead of 1024).
            csb = sp.tile([128, N], BF, tag="csb")
            nc.vector.tensor_copy(csb[:], cs[:])
            sr = sp.tile([128, 8], FP, tag="sr")
            for c in range(8):
                ptr = pst.tile([128, 128], BF, tag="tr")
                nc.tensor.transpose(ptr[:], csb[:, c * 128:(c + 1) * 128],
                                    identb[:])
                with nc.allow_low_precision(reason="softmax denom, bf16 in"):
                    nc.vector.reciprocal(sr[:, c:c + 1], ptr[:, 0:1])
            sr64 = sp.tile([128, 8], FP, tag="sr64")
            nc.vector.tensor_scalar_mul(sr64[:], sr[:], 64.0)
            sr512 = sp.tile([128, 8], FP, tag="sr512")
            nc.gpsimd.tensor_scalar_mul(sr512[:], sr[:], 512.0)

            # late DMAs: not needed before S8+, keep head bandwidth clear
            wq8 = pp.tile([128, QO * 2 * 128], F8, tag="wq8")
            nc.sync.dma_start(wq8[:], wq8_d[:])
            wpb2 = pp.tile([128, QO * 2 * 128], BF, tag="wpb2")
            nc.sync.dma_start(wpb2[:], wpb2_d[:])
            bias512 = pp.tile([128, 1], FP, tag="bias512")
            nc.sync.dma_start(bias512[:], bias_d[:])
            xtt = pp.tile([128, DCH * N], BF, tag="xtt")
            nc.sync.dma_start(xtt[:], xtt_d[:])
            lt8 = pp.tile([128, NCH * N], F8, tag="lt8")
            nc.sync.dma_start(lt8[:], lt8_d[:])
            lt8v = lt8[:].rearrange("q (m n) -> q m n", m=NCH)
            l2t8 = pp.tile([128, NCH * N], F8, tag="l2t8")
            nc.sync.dma_start(l2t8[:], l2t8_d[:])
            l2t8v = l2t8[:].rearrange("q (m n) -> q m n", m=NCH)

            # zall: fp8 x512 copies of Z0^T / Z1^T for the tail DR pair
            zall = pp.tile([128, 2 * DCH * N], F8, tag="zall")
            zallv = zall[:].rearrange("q (k pn) -> q k pn", k=2)

            # ---- S8: Z0 NATURAL (n on partitions): psum = satb_un^T-contracted
            # with x_TA; per-partition drains apply the softmax scale free.
            z0nb = pp.tile([128, NCH * D], F8, tag="z0nb")
            z0nbv = z0nb[:].rearrange("q (m d) -> q m d", m=NCH)
            z0bf = pp.tile([128, NCH * D], BF, tag="z0bf")
            for i in range(NCH):
                pb = psb.tile([128, N], FP, tag="big")
                for c0, c1 in ((0, 512), (512, 768)):  # psum-bank-aligned
                    for a2 in range(4):  # DoubleRow over m-chunk pairs
                        nc.tensor.matmul(
                            pb[:, c0:c1],
                            satbv[:, 2 * a2:2 * a2 + 2, i * 128:(i + 1) * 128],
                            anbv[:, 2 * a2:2 * a2 + 2, c0:c1],
                            start=(a2 == 0), stop=(a2 == 3), perf_mode=DR)
                # z0nb = 64*Z0 (fp8, L-contraction; Act writes fp8 fast),
                # z0bf = 512*Z0 (bf16 for the tail transposes, on DVE)
                nc.scalar.activation(z0nb[:, i * D:(i + 1) * D], pb[:, 0:D],
                                     AF.Copy, scale=sr64[:, i:i + 1])
                nc.vector.tensor_scalar_mul(z0bf[:, i * D:(i + 1) * D],
                                            pb[:, 0:D], sr512[:, i:i + 1])

            # ---- S9': zall0 = transpose(z0bf) -> fp8 (already x512) ----
            for p in range(DCH):
                pz = pst.tile([128, N], BF, tag="tr")
                for i in range(NCH):
                    nc.tensor.transpose(
                        pz[:, i * 128:(i + 1) * 128],
                        z0bf[:, i * D + p * 128:i * D + (p + 1) * 128],
                        identb[:])
                copy_rr(zall[:, p * N:(p + 1) * N], pz[:])

            # ---- S10: Z1T = (L@Z0)^T, drained straight to fp8 x512 ----
            for p in range(DCH):
                pb = psb.tile([128, N], FP, tag="big")
                for h in range(2):
                    for a2 in range(4):
                        nc.tensor.matmul(
                            pb[:, h * 512:(h + 1) * 512],
                            z0nbv[:, 2 * a2:2 * a2 + 2, p * 128:(p + 1) * 128],
                            lt8v[:, 2 * a2:2 * a2 + 2, h * 512:(h + 1) * 512],
                            start=(a2 == 0), stop=(a2 == 3), perf_mode=DR)
                # psum holds 512*(L@Z0)^T == the x512 fp8 tail scale
                copy_rr(zall[:, (DCH + p) * N:(DCH + p + 1) * N], pb[:])

            # ---- S12': Z2T' = (2L^2 @ Z0)^T via host-precomputed L2 ----
            Z2T = []
            for p in range(DCH):
                pb = psb.tile([128, N], FP, tag="big")
                for h in range(2):
                    for a2 in range(4):
                        nc.tensor.matmul(
                            pb[:, h * 512:(h + 1) * 512],
                            z0nbv[:, 2 * a2:2 * a2 + 2, p * 128:(p + 1) * 128],
                            l2t8v[:, 2 * a2:2 * a2 + 2, h * 512:(h + 1) * 512],
                            start=(a2 == 0), stop=(a2 == 3), perf_mode=DR)
                t_ = pp.tile([128, N], BF, name=f"z2tZ{p}", tag=f"Z2{p}")
                scaled_rr(t_[:], pb[:], 1.0 / (64.0 * S2S))
                Z2T.append(t_)

            # ---- S13: projection; psum is 512-scaled, host divides out ----
            wq8v = wq8[:].rearrange("q (s k c) -> q s k c", s=QO, k=2)
            for q in range(QO):
                p = q // 2
                pb = psb.tile([128, N], FP, tag="big")
                for h in range(2):
                    nc.tensor.matmul(
                        pb[:, h * 512:(h + 1) * 512],
                        wq8v[:, q],
                        zallv[:, :, p * N + h * 512:p * N + h * 512 + 512],
                        start=True, stop=False, perf_mode=DR)
                    nc.tensor.matmul(
                        pb[:, h * 512:(h + 1) * 512],
                        wpb2[:, (2 * q) * 128:(2 * q + 1) * 128],
                        Z2T[p][:, h * 512:(h + 1) * 512],
                        start=False, stop=False)
                    nc.tensor.matmul(
                        pb[:, h * 512:(h + 1) * 512],
                        wpb2[:, (2 * q + 1) * 128:(2 * q + 2) * 128],
                        xtt[:, p * N + h * 512:p * N + (h + 1) * 512],
                        start=False, stop=True)
                ob = sp.tile([128, N], BF, tag="outbuf", bufs=2)
                if q == QO - 1:
                    # the last drain gates the final DMA + kernel end: split
                    # it across Act and DVE so it finishes in half the time
                    nc.scalar.activation(ob[:, 0:512], pb[:, 0:512], AF.Relu,
                                         bias=bias512[:])
                    nc.vector.tensor_scalar(ob[:, 512:N], pb[:, 512:N],
                                            bias512[:], 0.0,
                                            op0=OP.add, op1=OP.max)
                elif q % 2 == 0:
                    nc.scalar.activation(ob[:], pb[:], AF.Relu, bias=bias512[:])
                else:
                    nc.vector.tensor_scalar(ob[:], pb[:], bias512[:], 0.0,
                                            op0=OP.add, op1=OP.max)
                nc.sync.dma_start(out_d[q * 128:(q + 1) * 128, :], ob[:])

    nc.compile()
    _compiled["nc"] = nc
    return nc


def _blk128(a, nblk):
    """(nblk*128, C) -> (128, nblk*C): row-chunk i becomes col block i."""
    a = np.asarray(a)
    c = a.shape[1]
    return np.ascontiguousarray(
        a.reshape(nblk, 128, c).transpose(1, 0, 2).reshape(128, nblk * c))


def _host_prep(x, edge_index, edge_weight, Ve, be, Vs, bs, cheb_W, cheb_b, res_W, res_b):
    import ml_dtypes
    BF = ml_dtypes.bfloat16
    F8H = ml_dtypes.float8_e4m3
    row = np.asarray(edge_index[0]).astype(np.int64)
    col = np.asarray(edge_index[1]).astype(np.int64)
    w = np.asarray(edge_weight, np.float64).copy()
    w[row == col] = 0.0
    deg = np.zeros(N, np.float64)
    np.add.at(deg, row, w)
    dis = np.where(deg > 0, 1.0 / np.sqrt(np.where(deg > 0, deg, 1.0)), 0.0)
    norm = -dis[row] * w * dis[col]
    L = np.zeros((N, N), np.float64)
    np.add.at(L, (col, row), norm)
    L2 = 2.0 * (L @ L)

    cheb_W = np.asarray(cheb_W, np.float64)
    res_W = np.asarray(res_W, np.float64)
    W0p = cheb_W[0] - cheb_W[2]
    # fp8 DR pair blocks (k0=W0-W2 vs Z0, k1=W1 vs Z1) and bf16 blocks
    # (512*W2 vs Z2', 512*Wres vs X). Row embedding: block rows 32a..32a+32
    # hold W for out half b (t = 2q+b), a = b + (0 if q even else 2).
    wq8 = np.zeros((QO, 2, 128, 128), np.float64)
    wpb2 = np.zeros((QO, 2, 128, 128), np.float64)
    for q in range(QO):
        off = 0 if q % 2 == 0 else 2
        for b_ in range(2):
            a = b_ + off
            wq8[q, 0, 32 * a:32 * a + 32, 64 * b_:64 * b_ + 64] = W0p
            wq8[q, 1, 32 * a:32 * a + 32, 64 * b_:64 * b_ + 64] = cheb_W[1]
            wpb2[q, 0, 32 * a:32 * a + 32, 64 * b_:64 * b_ + 64] = 512.0 * cheb_W[2]
            wpb2[q, 1, 32 * a:32 * a + 32, 64 * b_:64 * b_ + 64] = 512.0 * res_W.T
    wq8 = np.ascontiguousarray(
        wq8.transpose(2, 0, 1, 3).reshape(128, QO * 2 * 128)).astype(F8H)
    wpb2 = np.ascontiguousarray(
        wpb2.transpose(2, 0, 1, 3).reshape(128, QO * 2 * 128)).astype(BF)

    b64 = 512.0 * (np.asarray(cheb_b, np.float64) + np.asarray(res_b, np.float64))
    bias512 = np.concatenate([b64, b64]).reshape(128, 1).astype(np.float32)

    bigi = np.zeros((128, 384), np.float32)
    bigi[np.arange(128), 128 + np.arange(128)] = 1.0
    return {
        "bigi": bigi.astype(BF),
        "identb": np.eye(128, dtype=np.float32).astype(BF),
        "ident8": np.eye(128, dtype=np.float32).astype(F8H),
        "vetb": np.ascontiguousarray(np.asarray(Ve, np.float32).T).astype(BF),
        "be": np.ascontiguousarray(np.asarray(be, np.float32)[0]).astype(BF),
        "vst8": _blk128(np.asarray(Vs, np.float64).T, NCH).astype(F8H),
        "bst": _blk128(np.asarray(bs, np.float64)[0].T, NCH).astype(BF),
        "lt8": _blk128(8.0 * L.T, NCH).astype(F8H),
        "l2t8": _blk128(S2S * L2.T, NCH).astype(F8H),
        "wq8": wq8,
        "wpb2": wpb2,
        "bias512": bias512,
    }


TRACE = False
LAST = {}


def kernel(x, edge_index, edge_weight, Ve, be, Vs, bs, cheb_W, cheb_b, res_W, res_b):
    from concourse.bass_utils import run_bass_kernel_spmd
    import ml_dtypes
    BF = ml_dtypes.bfloat16
    F8H = ml_dtypes.float8_e4m3

    x = np.asarray(x, np.float32)
    shared = _host_prep(x, edge_index, edge_weight, Ve, be, Vs, bs,
                        cheb_W, cheb_b, res_W, res_b)
    nc = _build()
    in_maps = []
    for b in range(B):
        m = dict(shared)
        xb = x[b]                                   # (N, F, T)
        xnp = np.zeros((N, F, 32), np.float32)      # col 32f+t, zero padded
        xnp[:, :, :T] = xb
        m["xnp"] = _blk128(xnp.reshape(N, 1024), NCH).astype(F8H)
        m["xt8f"] = _blk128(xb.reshape(N, D).T, DCH).astype(F8H)
        m["xtt"] = _blk128(
            xb.transpose(2, 1, 0).reshape(D, N), DCH).astype(BF)  # d' = t*32+f
        in_maps.append(m)
    res = run_bass_kernel_spmd(nc, in_maps, list(range(B)), trace=TRACE)
    LAST["res"] = res
    out = np.stack(
        [(r["out"].astype(np.float32) * (1.0 / 512.0)).reshape(T, G, N)
         .transpose(2, 1, 0) for r in res.results], axis=0)
    return out
